# revision 15
# baseline (speedup 1.0000x reference)
"""Camera back-projection (truncated depth field) Trainium2 kernel.

out[b,0,i,j,k] = relu(1 - 128*|depth[b,0,vi(j,k),ui(i,k)] - zc_k|) with
frustum/validity masking, where (u,v) are pinhole projections of the voxel
grid. 8 cores, 2 batches/core, pure data parallel.

Device pipeline (per batch, per 4-k chunk):
  stage A (PE): psA[r,(k,i)] = sum_c winT[c,r] * Q[c,(k,i)] = d(r,k,i) - zc_k
      winT is the fp16 hi/lo split of the 252^2 depth window (transposed);
      Q is a one-hot fp16 column-selection (ui) with two augmented rows
      carrying -zc_hi/-zc_lo. Exact to ~1e-6.
  tent (ACT/DVE): F[r,(k,i)] = relu(1 - 128*|psA|)  -> fp16 (err <= 2.5e-4)
  stage B (PE): psB[j,(k,i)] = sum_r P[r,(k,j)] * F[r,(k,i)]
      P is a one-hot fp16 row-selection (vi); invalid voxels have all-zero
      one-hot columns and come out exactly 0.
  drain (ACT/DVE): out_sb[j,(k,i)] = psB -> f32 -> contiguous DMA out.
Host: out[b,0,i,j,k] = outdev[b][j,k,i] (pure transpose).
"""
import sys
import numpy as np

sys.path.insert(0, "/opt/trn_rl_repo")

RES = 128
IMG = 480
N = 16
NCORES = 8
BPC = N // NCORES          # batches per core
WIN = 252                  # depth window rows/cols actually used
WPAD = 256                 # padded to 2 partition tiles
KCH = 4                    # k's per pipeline chunk
NCHUNK = RES // KCH
POISON = np.float32(100.0) # fp16-safe "far" depth for invalid samples

_nc_cache = {}


def _build_program():
    import concourse.bacc as bacc
    import concourse.mybir as mybir
    import concourse.tile as tile

    P = 128
    NF = KCH * RES             # free size per chunk (512)
    nc = bacc.Bacc(None, target_bir_lowering=False, debug=False)
    with tile.TileContext(nc) as tc:
        with tc.tile_pool(name="dram", bufs=1, space="DRAM") as dram:
            wts, qs, ps_, outs = {}, {}, {}, {}
            for b in range(BPC):
                for s in ("hi", "lo"):
                    wts[b, s] = dram.tile([2, P, WPAD], mybir.dt.float16,
                                          kind="ExternalInput", uniquify=False, name=f"wt_{s}{b}")
                qs[b] = dram.tile([P, NCHUNK * 4 * KCH * RES], mybir.dt.float16,
                                  kind="ExternalInput", uniquify=False, name=f"qp{b}")
                outs[b] = dram.tile([RES, RES * RES], mybir.dt.float32,
                                    kind="ExternalOutput", uniquify=False, name=f"outdev{b}")

            with (
                tc.tile_pool(name="sb", bufs=1) as sb,
                tc.tile_pool(name="ps", bufs=1, space="PSUM") as ps,
            ):
                for b in range(BPC):
                    wt_sb = {}
                    for s in ("hi", "lo"):
                        for c in range(2):
                            t = sb.tile([P, WPAD], mybir.dt.float16,
                                        name=f"wt_{s}{c}_{b}", tag=f"wt_{s}{c}", bufs=1)
                            nc.scalar.dma_start(t[:], wts[b, s][c])
                            wt_sb[s, c] = t

                    state = {}
                    for ch in range(NCHUNK + 1):
                        if ch < NCHUNK:
                            qp = sb.tile([P, 4 * NF], mybir.dt.float16,
                                         name=f"qp_{b}_{ch}", tag="qp", bufs=6)
                            nc.sync.dma_start(qp[:], qs[b][:, ch * 4 * NF:(ch + 1) * 4 * NF])
                            qc = {c: qp[:, c * NF:(c + 1) * NF] for c in range(2)}
                            pc = {rt: qp[:, (2 + rt) * NF:(3 + rt) * NF] for rt in range(2)}

                            psA = ps.tile([P, 2 * NF], mybir.dt.float32,
                                          name=f"psA_{b}_{ch}", tag="psA", bufs=3)
                            combos = [("hi", 0), ("hi", 1), ("lo", 0), ("lo", 1)]
                            for m, (s, c) in enumerate(combos):
                                for rt in range(2):
                                    nc.tensor.matmul(
                                        psA[:, rt * NF:(rt + 1) * NF],
                                        wt_sb[s, c][:, rt * P:(rt + 1) * P],
                                        qc[c],
                                        start=(m == 0), stop=(m == 3),
                                    )

                            # tent -> fp16 F; Abs on ACT, halves on ACT/DVE
                            F = {}
                            for rt in range(2):
                                F[rt] = sb.tile([P, NF], mybir.dt.float16,
                                                name=f"F{rt}_{b}_{ch}", tag=f"F{rt}", bufs=6)
                            aa = sb.tile([P, 2 * NF], mybir.dt.float32,
                                         name=f"aa_{b}_{ch}", tag="aa", bufs=4)
                            nc.scalar.activation(aa[:], psA[:],
                                                 mybir.ActivationFunctionType.Abs)
                            nc.scalar.activation(F[0][:], aa[:, :NF],
                                                 mybir.ActivationFunctionType.Relu,
                                                 bias=1.0, scale=-128.0)
                            t1 = sb.tile([P, NF], mybir.dt.float32,
                                         name=f"t1_{b}_{ch}", tag="t1", bufs=4)
                            nc.vector.tensor_scalar(t1[:], aa[:, NF:],
                                                    scalar1=-128.0, scalar2=1.0,
                                                    op0=mybir.AluOpType.mult,
                                                    op1=mybir.AluOpType.add)
                            nc.vector.tensor_scalar(F[1][:], t1[:],
                                                    scalar1=0.0, scalar2=None,
                                                    op0=mybir.AluOpType.max)
                            state[ch] = (pc, F)

                        # stage B one chunk behind, so PE never waits on tent
                        pch = ch - 1
                        if pch >= 0:
                            pcp, Fp = state.pop(pch)
                            psB = ps.tile([P, NF], mybir.dt.float32,
                                          name=f"psB_{b}_{pch}", tag="psB", bufs=2)
                            for kc in range(KCH):
                                ksl = slice(kc * RES, (kc + 1) * RES)
                                for rt in range(2):
                                    nc.tensor.matmul(
                                        psB[:, ksl],
                                        pcp[rt][:, ksl],
                                        Fp[rt][:, ksl],
                                        start=(rt == 0), stop=(rt == 1),
                                    )
                            ob = sb.tile([P, NF], mybir.dt.float32,
                                         name=f"ob_{b}_{pch}", tag="ob", bufs=4)
                            nc.vector.tensor_copy(ob[:], psB[:])
                            nc.gpsimd.dma_start(outs[b][:, pch * NF:(pch + 1) * NF], ob[:])
    nc.compile()
    return nc


def _host_precompute(depth, fl, cd):
    """Per-batch device inputs. Index math in float32, matching the jax
    reference op-for-op."""
    f32 = np.float32
    res = RES
    c = ((np.arange(res, dtype=f32) + f32(0.5)) / f32(res)) - f32(0.5)
    zc = f32(cd) - c                        # [k]
    kvalid = zc > 0
    with np.errstate(divide="ignore", invalid="ignore"):
        u = (f32(fl) * c)[:, None] / zc[None, :] + f32((IMG - 1) * 0.5)  # [i,k] == [j,k]
    ui = np.clip(np.round(u), 0, IMG - 1).astype(np.int64)
    mu = (u >= 0) & (u <= IMG - 1) & kvalid[None, :]

    if mu.any():
        cmin = int(ui[mu].min())
        cmax = int(ui[mu].max())
    else:
        cmin = cmax = 0
    if (cmax - cmin) >= WIN:
        raise NotImplementedError("projection span exceeds window")
    base = min(cmin, IMG - WIN)   # window base for both rows and cols (u==v)

    w = depth[base:base + WIN, base:base + WIN].astype(f32).copy()
    w[w <= 0] = POISON
    wpad = np.zeros((WPAD, WPAD), dtype=f32)
    wpad[:WIN, :WIN] = w
    w_hi = wpad.astype(np.float16)
    w_lo = (wpad - w_hi.astype(f32)).astype(np.float16)
    # winT[c, r] tiles [2, 128, 256]; aug rows at c=254,255 (hi=1.0) carry -zc
    wt_hi = np.ascontiguousarray(w_hi.T).reshape(2, 128, WPAD)
    wt_lo = np.ascontiguousarray(w_lo.T).reshape(2, 128, WPAD)
    wt_hi[1, 126, :] = np.float16(1.0)
    wt_hi[1, 127, :] = np.float16(1.0)
    wt_lo[1, 126:, :] = 0

    nzc = -zc
    nzc_hi = nzc.astype(np.float16)
    nzc_lo = (nzc - nzc_hi.astype(f32)).astype(np.float16)

    # Q[c, (k,i)]: one-hot ui, plus aug rows
    q = np.zeros((2, 128, res * res), dtype=np.float16)
    ii, kk = np.nonzero(mu)
    cloc = (ui[ii, kk] - base).astype(np.int64)
    q[cloc // 128, cloc % 128, kk * res + ii] = np.float16(1.0)
    q[1, 126, :] = np.repeat(np.where(kvalid, nzc_hi, np.float16(0)), res)
    q[1, 127, :] = np.repeat(np.where(kvalid, nzc_lo, np.float16(0)), res)

    # P[r, (k,j)]: one-hot vi (v == u maps with j in place of i)
    p = np.zeros((2, 128, res * res), dtype=np.float16)
    p[cloc // 128, cloc % 128, kk * res + ii] = np.float16(1.0)

    # interleave per-chunk so one DMA per chunk fetches Q tiles + P tiles
    nf = KCH * res
    qp = np.empty((128, NCHUNK, 4, nf), dtype=np.float16)
    qv = q.reshape(2, 128, NCHUNK, nf)
    pv = p.reshape(2, 128, NCHUNK, nf)
    qp[:, :, 0] = qv[0]
    qp[:, :, 1] = qv[1]
    qp[:, :, 2] = pv[0]
    qp[:, :, 3] = pv[1]
    return wt_hi, wt_lo, qp.reshape(128, -1)


def kernel(depth_t, fl, cam_dist):
    from concourse.bass_utils import run_bass_kernel_spmd

    depth_t = np.asarray(depth_t)
    fl = np.asarray(fl).reshape(N)
    cam_dist = np.asarray(cam_dist).reshape(N)

    if "nc" not in _nc_cache:
        _nc_cache["nc"] = _build_program()
    nc = _nc_cache["nc"]

    cache = {}
    in_maps = []
    for core in range(NCORES):
        m = {}
        for b in range(BPC):
            g = core * BPC + b
            key = (float(fl[g]), float(cam_dist[g]), g)
            wt_hi, wt_lo, qp = _host_precompute(depth_t[g, 0], fl[g], cam_dist[g])
            m[f"wt_hi{b}"] = wt_hi
            m[f"wt_lo{b}"] = wt_lo
            m[f"qp{b}"] = qp
        in_maps.append(m)

    globals()["_last_in_maps"] = in_maps
    r = run_bass_kernel_spmd(nc, in_maps, list(range(NCORES)))

    out = np.empty((N, 1, RES, RES, RES), dtype=np.float32)
    for core in range(NCORES):
        for b in range(BPC):
            g = core * BPC + b
            od = r.results[core][f"outdev{b}"].reshape(RES, RES, RES)  # [j,k,i]
            out[g, 0] = od.transpose(2, 0, 1)
    return out


# revision 16
# speedup vs baseline: 1.0240x; 1.0240x over previous
"""Camera back-projection (truncated depth field) Trainium2 kernel.

out[b,0,i,j,k] = relu(1 - 128*|depth[b,0,vi(j,k),ui(i,k)] - zc_k|) with
frustum/validity masking, where (u,v) are pinhole projections of the voxel
grid. 8 cores, 2 batches/core, pure data parallel.

Device pipeline (per batch, per 4-k chunk):
  stage A (PE): psA[r,(k,i)] = sum_c winT[c,r] * Q[c,(k,i)] = d(r,k,i) - zc_k
      winT is the fp16 hi/lo split of the 252^2 depth window (transposed);
      Q is a one-hot fp16 column-selection (ui) with two augmented rows
      carrying -zc_hi/-zc_lo. Exact to ~1e-6.
  tent (ACT/DVE): F[r,(k,i)] = relu(1 - 128*|psA|)  -> fp16 (err <= 2.5e-4)
  stage B (PE): psB[j,(k,i)] = sum_r P[r,(k,j)] * F[r,(k,i)]
      P is a one-hot fp16 row-selection (vi); invalid voxels have all-zero
      one-hot columns and come out exactly 0.
  drain (ACT/DVE): out_sb[j,(k,i)] = psB -> f32 -> contiguous DMA out.
Host: out[b,0,i,j,k] = outdev[b][j,k,i] (pure transpose).
"""
import sys
import numpy as np

sys.path.insert(0, "/opt/trn_rl_repo")

RES = 128
IMG = 480
N = 16
NCORES = 8
BPC = N // NCORES          # batches per core
WIN = 252                  # depth window rows/cols actually used
WPAD = 256                 # padded to 2 partition tiles
KCH = 4                    # k's per pipeline chunk
NCHUNK = RES // KCH
POISON = np.float32(100.0) # fp16-safe "far" depth for invalid samples

_nc_cache = {}


def _build_program():
    import concourse.bacc as bacc
    import concourse.mybir as mybir
    import concourse.tile as tile

    P = 128
    NF = KCH * RES             # free size per chunk (512)
    nc = bacc.Bacc(None, target_bir_lowering=False, debug=False)
    with tile.TileContext(nc) as tc:
        with tc.tile_pool(name="dram", bufs=1, space="DRAM") as dram:
            wts, qs, ps_, outs = {}, {}, {}, {}
            for b in range(BPC):
                for s in ("hi", "lo"):
                    wts[b, s] = dram.tile([2, P, WPAD], mybir.dt.float16,
                                          kind="ExternalInput", uniquify=False, name=f"wt_{s}{b}")
                qs[b] = dram.tile([P, NCHUNK * 4 * KCH * RES], mybir.dt.float16,
                                  kind="ExternalInput", uniquify=False, name=f"qp{b}")
                outs[b] = dram.tile([RES, RES * RES], mybir.dt.float32,
                                    kind="ExternalOutput", uniquify=False, name=f"outdev{b}")

            with (
                tc.tile_pool(name="sb", bufs=1) as sb,
                tc.tile_pool(name="ps", bufs=1, space="PSUM") as ps,
            ):
                for b in range(BPC):
                    wt_sb = {}
                    for s in ("hi", "lo"):
                        for c in range(2):
                            t = sb.tile([P, WPAD], mybir.dt.float16,
                                        name=f"wt_{s}{c}_{b}", tag=f"wt_{s}{c}", bufs=2)
                            nc.sync.dma_start(t[:], wts[b, s][c])
                            wt_sb[s, c] = t

                    state = {}
                    for ch in range(NCHUNK + 1):
                        if ch < NCHUNK:
                            qp = sb.tile([P, 4 * NF], mybir.dt.float16,
                                         name=f"qp_{b}_{ch}", tag="qp", bufs=6)
                            nc.sync.dma_start(qp[:], qs[b][:, ch * 4 * NF:(ch + 1) * 4 * NF])
                            qc = {c: qp[:, c * NF:(c + 1) * NF] for c in range(2)}
                            pc = {rt: qp[:, (2 + rt) * NF:(3 + rt) * NF] for rt in range(2)}

                            psA = ps.tile([P, 2 * NF], mybir.dt.float32,
                                          name=f"psA_{b}_{ch}", tag="psA", bufs=3)
                            combos = [("hi", 0), ("hi", 1), ("lo", 0), ("lo", 1)]
                            for m, (s, c) in enumerate(combos):
                                for rt in range(2):
                                    nc.tensor.matmul(
                                        psA[:, rt * NF:(rt + 1) * NF],
                                        wt_sb[s, c][:, rt * P:(rt + 1) * P],
                                        qc[c],
                                        start=(m == 0), stop=(m == 3),
                                    )

                            # tent -> fp16 F; Abs on ACT, halves on ACT/DVE
                            F = {}
                            for rt in range(2):
                                F[rt] = sb.tile([P, NF], mybir.dt.float16,
                                                name=f"F{rt}_{b}_{ch}", tag=f"F{rt}", bufs=6)
                            aa = sb.tile([P, 2 * NF], mybir.dt.float32,
                                         name=f"aa_{b}_{ch}", tag="aa", bufs=4)
                            nc.scalar.activation(aa[:], psA[:],
                                                 mybir.ActivationFunctionType.Abs)
                            nc.scalar.activation(F[0][:], aa[:, :NF],
                                                 mybir.ActivationFunctionType.Relu,
                                                 bias=1.0, scale=-128.0)
                            t1 = sb.tile([P, NF], mybir.dt.float32,
                                         name=f"t1_{b}_{ch}", tag="t1", bufs=4)
                            nc.vector.tensor_scalar(t1[:], aa[:, NF:],
                                                    scalar1=-128.0, scalar2=1.0,
                                                    op0=mybir.AluOpType.mult,
                                                    op1=mybir.AluOpType.add)
                            nc.vector.tensor_scalar(F[1][:], t1[:],
                                                    scalar1=0.0, scalar2=None,
                                                    op0=mybir.AluOpType.max)
                            state[ch] = (pc, F)

                        # stage B one chunk behind, so PE never waits on tent
                        pch = ch - 1
                        if pch >= 0:
                            pcp, Fp = state.pop(pch)
                            psB = ps.tile([P, NF], mybir.dt.float32,
                                          name=f"psB_{b}_{pch}", tag="psB", bufs=2)
                            for kc in range(KCH):
                                ksl = slice(kc * RES, (kc + 1) * RES)
                                for rt in range(2):
                                    nc.tensor.matmul(
                                        psB[:, ksl],
                                        pcp[rt][:, ksl],
                                        Fp[rt][:, ksl],
                                        start=(rt == 0), stop=(rt == 1),
                                    )
                            ob = sb.tile([P, NF], mybir.dt.float32,
                                         name=f"ob_{b}_{pch}", tag="ob", bufs=4)
                            nc.vector.tensor_copy(ob[:], psB[:])
                            nc.gpsimd.dma_start(outs[b][:, pch * NF:(pch + 1) * NF], ob[:])
    nc.compile()
    return nc


def _host_precompute(depth, fl, cd):
    """Per-batch device inputs. Index math in float32, matching the jax
    reference op-for-op."""
    f32 = np.float32
    res = RES
    c = ((np.arange(res, dtype=f32) + f32(0.5)) / f32(res)) - f32(0.5)
    zc = f32(cd) - c                        # [k]
    kvalid = zc > 0
    with np.errstate(divide="ignore", invalid="ignore"):
        u = (f32(fl) * c)[:, None] / zc[None, :] + f32((IMG - 1) * 0.5)  # [i,k] == [j,k]
    ui = np.clip(np.round(u), 0, IMG - 1).astype(np.int64)
    mu = (u >= 0) & (u <= IMG - 1) & kvalid[None, :]

    if mu.any():
        cmin = int(ui[mu].min())
        cmax = int(ui[mu].max())
    else:
        cmin = cmax = 0
    if (cmax - cmin) >= WIN:
        raise NotImplementedError("projection span exceeds window")
    base = min(cmin, IMG - WIN)   # window base for both rows and cols (u==v)

    w = depth[base:base + WIN, base:base + WIN].astype(f32).copy()
    w[w <= 0] = POISON
    wpad = np.zeros((WPAD, WPAD), dtype=f32)
    wpad[:WIN, :WIN] = w
    w_hi = wpad.astype(np.float16)
    w_lo = (wpad - w_hi.astype(f32)).astype(np.float16)
    # winT[c, r] tiles [2, 128, 256]; aug rows at c=254,255 (hi=1.0) carry -zc
    wt_hi = np.ascontiguousarray(w_hi.T).reshape(2, 128, WPAD)
    wt_lo = np.ascontiguousarray(w_lo.T).reshape(2, 128, WPAD)
    wt_hi[1, 126, :] = np.float16(1.0)
    wt_hi[1, 127, :] = np.float16(1.0)
    wt_lo[1, 126:, :] = 0

    nzc = -zc
    nzc_hi = nzc.astype(np.float16)
    nzc_lo = (nzc - nzc_hi.astype(f32)).astype(np.float16)

    # Q[c, (k,i)]: one-hot ui, plus aug rows
    q = np.zeros((2, 128, res * res), dtype=np.float16)
    ii, kk = np.nonzero(mu)
    cloc = (ui[ii, kk] - base).astype(np.int64)
    q[cloc // 128, cloc % 128, kk * res + ii] = np.float16(1.0)
    q[1, 126, :] = np.repeat(np.where(kvalid, nzc_hi, np.float16(0)), res)
    q[1, 127, :] = np.repeat(np.where(kvalid, nzc_lo, np.float16(0)), res)

    # P[r, (k,j)]: one-hot vi (v == u maps with j in place of i)
    p = np.zeros((2, 128, res * res), dtype=np.float16)
    p[cloc // 128, cloc % 128, kk * res + ii] = np.float16(1.0)

    # interleave per-chunk so one DMA per chunk fetches Q tiles + P tiles
    nf = KCH * res
    qp = np.empty((128, NCHUNK, 4, nf), dtype=np.float16)
    qv = q.reshape(2, 128, NCHUNK, nf)
    pv = p.reshape(2, 128, NCHUNK, nf)
    qp[:, :, 0] = qv[0]
    qp[:, :, 1] = qv[1]
    qp[:, :, 2] = pv[0]
    qp[:, :, 3] = pv[1]
    return wt_hi, wt_lo, qp.reshape(128, -1)


def kernel(depth_t, fl, cam_dist):
    from concourse.bass_utils import run_bass_kernel_spmd

    depth_t = np.asarray(depth_t)
    fl = np.asarray(fl).reshape(N)
    cam_dist = np.asarray(cam_dist).reshape(N)

    if "nc" not in _nc_cache:
        _nc_cache["nc"] = _build_program()
    nc = _nc_cache["nc"]

    cache = {}
    in_maps = []
    for core in range(NCORES):
        m = {}
        for b in range(BPC):
            g = core * BPC + b
            key = (float(fl[g]), float(cam_dist[g]), g)
            wt_hi, wt_lo, qp = _host_precompute(depth_t[g, 0], fl[g], cam_dist[g])
            m[f"wt_hi{b}"] = wt_hi
            m[f"wt_lo{b}"] = wt_lo
            m[f"qp{b}"] = qp
        in_maps.append(m)

    globals()["_last_in_maps"] = in_maps
    r = run_bass_kernel_spmd(nc, in_maps, list(range(NCORES)))

    out = np.empty((N, 1, RES, RES, RES), dtype=np.float32)
    for core in range(NCORES):
        for b in range(BPC):
            g = core * BPC + b
            od = r.results[core][f"outdev{b}"].reshape(RES, RES, RES)  # [j,k,i]
            out[g, 0] = od.transpose(2, 0, 1)
    return out
